# revision 20
# baseline (speedup 1.0000x reference)
"""CIN (xDeepFM compressed interaction network) kernel for Trainium2.

Reference computation (per batch b, embedding dim d):
  h1[b,h,d] = sum_{i,j} x[b,i,d] * x[b,j,d]  * W0[i*39+j, h]   i,j < 39
  h2[b,h,d] = sum_{i,j} x[b,i,d] * h1[b,j,d] * W1[i*128+j, h]  i < 39, j < 128
  h3[b,h,d] = sum_{i,j} x[b,i,d] * h2[b,j,d] * W2[i*128+j, h]
  out[b, :] = concat(sum_d h1, sum_d h2, sum_d h3)   -> [B, 384]

Strategy: data-parallel over batch on 8 cores (512 batches each). On-chip
layout is feature-on-partitions / (b,d)-on-free-dim, fp16 compute with
fp32 PSUM accumulation, fully fused (h1/h2 never touch HBM).

Layer 1: the 780 upper-triangle pair products x_i*x_j are precomputed on
the host (packed [98, 8]), so layer 1 is 8 dense K=98 matmul passes.

Layer 2 needs z2[j,i,n] = x[i,n]*h1[j,n] with j on partitions: x rows
must be replicated across all 128 partitions (~5.1 MB/tile).  The
replica-write work is split between the 16 SDMA engines (21 rows via a
2-group DMA broadcast) and gpsimd partition_broadcast (18 rows).  The
gpsimd rows are bit-packed as fp32 pairs (the Q7 broadcast kernel moves
32-bit granules, so fp32 views double its byte rate to ~0.4 us/row).
All inputs are prefetched two tiles ahead so DVE z2 tiles for tile t+1
are fully built while the PE runs tile t, keeping the Tensor engine in
long back-to-back bursts (its clock ramps 1.2->2.4 GHz only after ~3 us
of continuous execution, so gaps are doubly expensive).

Layer 3 is never materialized: only sum_d h3 is needed, so per-batch
Gram matrices G2[b] = h2_b x_b^T are formed (PE transposes of h2 + K=64
matmuls against a host-pretransposed x^T; PE operands must start at
partition 0 -- partition-offset operands crash the HW) and contracted
with W2 every 16 tiles from an SBUF chunk accumulator.

sum_d h1 rides on the scalar engine's activation accumulate during the
PSUM->SBUF copies.  sum_d h2 is free on the PE: the host appends a
ones-column to the pre-transposed x^T gram operand, so each per-batch
gram matmul h2_b @ [x_b^T, 1] also emits the layer-2 output sums.
"""

import sys

sys.path.insert(0, "/opt/trn_rl_repo")

import numpy as np

M = 39          # fields
D = 64          # embedding dim
H = 128         # hidden per CIN layer
B_TOTAL = 4096
N_CORES = 8
B_CORE = B_TOTAL // N_CORES      # 512 batches per core
TILE_B = 8                       # batches per tile
TILE_N = TILE_B * D              # 512 columns per tile
L1_CHUNK = 98                    # partition rows for layer-1 pair products
L1_K = 8                         # pair slots per row (98*8 = 784 >= 780)
ROWS_PE = 18                     # x-replica rows built by gpsimd broadcast
ROWS_DMA = M - ROWS_PE
CHUNK_T = 16                     # tiles per layer-3 output chunk
N_CHUNKS = 4

_NC_CACHE = {}

# upper-triangle pairs (i <= j), row-major packed into [98, 8]
_PAIRS = [(i, j) for j in range(M) for i in range(j + 1)]
assert len(_PAIRS) == 780


def _build(b_core):
    import concourse.bacc as bacc
    import concourse.tile as tile
    from concourse import mybir
    from concourse.masks import make_identity

    f32 = mybir.dt.float32
    f16 = mybir.dt.float16

    n_tiles = b_core // TILE_B

    nc = bacc.Bacc("TRN2", target_bir_lowering=False, debug=False)
    # host-prepared tensors (fp16, pre-arranged); see kernel() below
    xt16_d = nc.dram_tensor(
        "xt16", [n_tiles, M, TILE_N], f16, kind="ExternalInput"
    )
    z1_d = nc.dram_tensor(
        "z1p", [n_tiles, L1_CHUNK, L1_K, TILE_N], f16, kind="ExternalInput"
    )
    xd_d = nc.dram_tensor(
        "xdt", [n_tiles, D, TILE_B, M + 1], f16, kind="ExternalInput"
    )
    w0_d = nc.dram_tensor(
        "W0s", [L1_CHUNK, L1_K, H], f16, kind="ExternalInput"
    )
    w1_d = nc.dram_tensor("W1t", [H, M, H], f16, kind="ExternalInput")
    w2_d = nc.dram_tensor("W2t", [H, M, H], f16, kind="ExternalInput")
    out_d = nc.dram_tensor("out", [3, H, b_core], f32, kind="ExternalOutput")

    # bcast DMA row groups (rows ROWS_DMA.. come from gpsimd broadcast);
    # z2 multiply groups cover all M rows
    DMA_GRPS = [(0, 11), (11, 10)]
    Z2_GRPS = [(0, 10), (10, 10), (20, 10), (30, 9)]

    with tile.TileContext(nc) as tc:
        with tc.tile_pool(name="resident", bufs=1) as resident:
            w0_sb = resident.tile([L1_CHUNK, L1_K, H], f16)
            nc.sync.dma_start(w0_sb[:], w0_d.ap())
            w1_sb = resident.tile([H, M, H], f16)
            nc.sync.dma_start(w1_sb[:], w1_d.ap())
            w2_sb = resident.tile([H, M, H], f16)
            nc.sync.dma_start(w2_sb[:], w2_d.ap())
            identity = resident.tile([H, H], f16)
            make_identity(nc, identity[:])

            # per-core accumulated outputs
            out_sb = resident.tile([H, b_core], f32)
            out2_sb = resident.tile([H, b_core], f32)
            out3_sb = resident.tile([H, b_core], f32)
            # layer-3 gram accumulator for the current 128-batch chunk
            g2t_sb = resident.tile([H, M, CHUNK_T * TILE_B], f16)

            xt16_ap = xt16_d.ap()  # [n_tiles, M, TILE_N], tile-major
            with (
                tc.tile_pool(name="bc", bufs=2) as bcp,
                tc.tile_pool(name="z1", bufs=3) as z1p,
                tc.tile_pool(name="xd", bufs=4) as xdp,
                tc.tile_pool(name="zpool", bufs=4) as zpool,
                tc.tile_pool(name="hsb", bufs=2) as hsb,
                tc.tile_pool(name="gram", bufs=1) as gram,
                tc.tile_pool(name="psum", bufs=2, space="PSUM") as psum,
                tc.tile_pool(name="psum_t", bufs=1, space="PSUM") as psum_t,
                tc.tile_pool(name="ps_o3", bufs=1, space="PSUM") as ps_o3,
            ):
                def issue_small_dmas(t):
                    z1h = z1p.tile([L1_CHUNK, L1_K, TILE_N], f16, tag="z1")
                    nc.sync.dma_start(z1h[:], z1_d.ap()[t])
                    xd_t = xdp.tile([D, TILE_B, M + 1], f16, tag="xd")
                    nc.sync.dma_start(xd_t[:], xd_d.ap()[t])
                    return z1h, xd_t

                def issue_bcast(t):
                    """x-replica tile: DMA rows 0..ROWS_DMA-1 (group-wise so
                    the z2 build can start early).  Rows ROWS_DMA.. are
                    seeded onto partition 0 by a small DMA and replicated
                    in-place by gpsimd (bit-packed fp32 for 2x Q7 rate)."""
                    bcast = bcp.tile([H, M, TILE_N], f16, tag="bc")
                    for i0, g in DMA_GRPS:
                        nc.sync.dma_start(
                            bcast[:, i0 : i0 + g, :],
                            xt16_ap[t][i0 : i0 + g]
                            .rearrange("i c -> (i c)")[None]
                            .to_broadcast([H, g * TILE_N]),
                        )
                    nc.sync.dma_start(
                        bcast[0:1, ROWS_DMA:, :],
                        xt16_ap[t][ROWS_DMA:]
                        .rearrange("i c -> (i c)")[None],
                    )
                    nc.gpsimd.partition_broadcast(
                        bcast[:, ROWS_DMA:, :].bitcast(f32),
                        bcast[0:1, ROWS_DMA:, :]
                        .rearrange("p i c -> p (i c)")
                        .bitcast(f32),
                    )
                    return bcast

                def layer1(t, z1h):
                    """h1 for tile t + its sum-over-d output accumulation."""
                    h1_ps = psum.tile([H, TILE_N], f32, tag="h1ps")
                    for k in range(L1_K):
                        nc.tensor.matmul(
                            h1_ps[:],
                            w0_sb[:, k, :],
                            z1h[:, k, :],
                            start=(k == 0),
                            stop=(k == L1_K - 1),
                        )
                    h1_16 = hsb.tile([H, TILE_N], f16, tag="h1")
                    for b in range(TILE_B):
                        bs = slice(b * D, (b + 1) * D)
                        nc.scalar.activation(
                            h1_16[:, bs],
                            h1_ps[:, bs],
                            mybir.ActivationFunctionType.Copy,
                            accum_out=out_sb[
                                :, t * TILE_B + b : t * TILE_B + b + 1
                            ],
                        )
                    return h1_16

                def z2_build(bcast, h1_16):
                    """z2 = x-replica * h1 (separate tiles keep DVE 2x)."""
                    z2g = []
                    for i0, g in Z2_GRPS:
                        z2 = zpool.tile([H, 10, TILE_N], f16, tag="z2")
                        nc.vector.tensor_mul(
                            z2[:, :g, :],
                            bcast[:, i0 : i0 + g, :],
                            h1_16[:, None, :].broadcast_to([H, g, TILE_N]),
                        )
                        z2g.append(z2)
                    return z2g

                def layer2(t, z2g):
                    h2_ps = psum.tile([H, TILE_N], f32, tag="h2ps")
                    for gi, (i0, g) in enumerate(Z2_GRPS):
                        for u in range(g):
                            i = i0 + u
                            nc.tensor.matmul(
                                h2_ps[:],
                                w1_sb[:, i, :],
                                z2g[gi][:, u, :],
                                start=(i == 0),
                                stop=(i == M - 1),
                            )
                    # sum_d h2 comes from the gram matmul's ones-column, so
                    # this is one plain PSUM->SBUF conversion copy
                    h2_16 = hsb.tile([H, TILE_N], f16, tag="h2")
                    nc.scalar.activation(
                        h2_16[:],
                        h2_ps[:],
                        mybir.ActivationFunctionType.Copy,
                    )
                    return h2_16

                def gram_transposes(t, h2_16):
                    # burst: all 8 per-batch h2 transposes into one PSUM tile
                    h2dt_ps = psum_t.tile([D, TILE_B, H], f16, tag="h2dtps")
                    for b in range(TILE_B):
                        bs = slice(b * D, (b + 1) * D)
                        nc.tensor.transpose(
                            h2dt_ps[:, b, :], h2_16[:, bs], identity[:]
                        )
                    h2dt = gram.tile([D, TILE_B, H], f16, tag="h2dt")
                    nc.scalar.copy(h2dt[:], h2dt_ps[:])
                    return h2dt

                def gram_matmuls(t, xd_t, h2dt):
                    # burst: all 8 per-batch gram matmuls; column M of the
                    # xd operand is ones, so output column M = sum_d h2
                    g2t_ps = psum_t.tile([H, TILE_B, M + 1], f32, tag="g2tps")
                    for b in range(TILE_B):
                        nc.tensor.matmul(
                            g2t_ps[:, b, :],
                            h2dt[:, b, :],
                            xd_t[:, b, :],
                            start=True,
                            stop=True,
                        )
                    off = (t % CHUNK_T) * TILE_B
                    nc.scalar.copy(
                        g2t_sb[:, :, off : off + TILE_B],
                        g2t_ps[:, :, :M].rearrange("p b i -> p i b"),
                    )
                    nc.scalar.copy(
                        out2_sb[:, t * TILE_B : (t + 1) * TILE_B],
                        g2t_ps[:, :, M],
                    )

                def final_chunk(ci):
                    cs = slice(ci * CHUNK_T * TILE_B, (ci + 1) * CHUNK_T * TILE_B)
                    for i in range(M):
                        nc.tensor.matmul(
                            out3_ps[:, cs],
                            w2_sb[:, i, :],
                            g2t_sb[:, i, :],
                            start=(i == 0),
                            stop=(i == M - 1),
                        )

                out3_ps = ps_o3.tile([H, b_core], f32)

                # ---- prologue: tiles 0/1 inputs; z2(0) fully built ----
                sm = [issue_small_dmas(0), issue_small_dmas(1)]
                bc = [issue_bcast(0), issue_bcast(1)]
                h1_c = layer1(0, sm[0][0])
                z2_c = z2_build(bc[0], h1_c)

                prev_gram = None
                for t in range(n_tiles):
                    # gram transposes of t-1 first: operands long ready, so
                    # the PE starts the tile with a dense burst
                    if prev_gram is not None:
                        h2dt_p = gram_transposes(prev_gram[0], prev_gram[2])

                    if t + 2 < n_tiles:
                        sm_nn = issue_small_dmas(t + 2)
                        bc_nn = issue_bcast(t + 2)
                    if t + 1 < n_tiles:
                        h1_n = layer1(t + 1, sm[1][0])
                        z2_n = z2_build(bc[1], h1_n)

                    h2_c = layer2(t, z2_c)

                    if prev_gram is not None:
                        gram_matmuls(prev_gram[0], prev_gram[1], h2dt_p)
                        if (t % CHUNK_T) == 0 and t > 0:
                            final_chunk(t // CHUNK_T - 1)
                    prev_gram = (t, sm[0][1], h2_c)

                    if t + 1 < n_tiles:
                        z2_c = z2_n
                        h1_c = h1_n
                        sm[0] = sm[1]
                        bc[0] = bc[1]
                        sm[1] = sm_nn if t + 2 < n_tiles else None
                        bc[1] = bc_nn if t + 2 < n_tiles else None

                h2dt_p = gram_transposes(prev_gram[0], prev_gram[2])
                gram_matmuls(prev_gram[0], prev_gram[1], h2dt_p)
                final_chunk(N_CHUNKS - 1)
                nc.vector.tensor_copy(out3_sb[:], out3_ps[:])

            nc.sync.dma_start(out_d.ap()[0], out_sb[:])
            nc.sync.dma_start(out_d.ap()[1], out2_sb[:])
            nc.sync.dma_start(out_d.ap()[2], out3_sb[:])
    nc.compile()
    return nc


def _get_nc(b_core):
    if b_core not in _NC_CACHE:
        _NC_CACHE[b_core] = _build(b_core)
    return _NC_CACHE[b_core]


_IDX = None


def _pair_index():
    """Flat [98*8] (i, j) index arrays; pad slots use (0, 0) with zero
    weights."""
    global _IDX
    if _IDX is None:
        ii = np.zeros(L1_CHUNK * L1_K, np.int64)
        jj = np.zeros(L1_CHUNK * L1_K, np.int64)
        for n, (i, j) in enumerate(_PAIRS):
            ii[n], jj[n] = i, j
        _IDX = (ii, jj)
    return _IDX


def _pack_weights(W0, W1, W2):
    w0r = W0.reshape(M, M, H).astype(np.float32)
    w0s = np.zeros((L1_CHUNK * L1_K, H), np.float32)
    for n, (i, j) in enumerate(_PAIRS):
        w0s[n] = w0r[i, j] + (w0r[j, i] if i != j else 0.0)
    w0s = w0s.reshape(L1_CHUNK, L1_K, H).astype(np.float16)
    w1t = np.ascontiguousarray(
        W1.reshape(M, H, H).transpose(1, 0, 2)
    ).astype(np.float16)
    w2t = np.ascontiguousarray(
        W2.reshape(M, H, H).transpose(1, 0, 2)
    ).astype(np.float16)
    return w0s, w1t, w2t


def _host_pack_core(xc, w0s, w1t, w2t):
    """Per-core input map; xc is [B_CORE, M, D] fp32."""
    n_tiles = B_CORE // TILE_B
    ii, jj = _pair_index()
    xtr32 = xc.transpose(1, 0, 2).reshape(M, n_tiles, TILE_N)
    xt32t = np.ascontiguousarray(xtr32.transpose(1, 0, 2))
    xt16t = xt32t.astype(np.float16)  # [n_tiles, M, TILE_N]
    z1 = np.ascontiguousarray(
        (xt32t[:, ii, :] * xt32t[:, jj, :])
        .astype(np.float16)
        .reshape(n_tiles, L1_CHUNK, L1_K, TILE_N)
    )
    xd = np.empty((n_tiles, D, TILE_B, M + 1), np.float16)
    xd[:, :, :, :M] = xt16t.reshape(n_tiles, M, TILE_B, D).transpose(0, 3, 2, 1)
    xd[:, :, :, M] = 1.0
    return {
        "xt16": xt16t,
        "z1p": z1,
        "xdt": xd,
        "W0s": w0s,
        "W1t": w1t,
        "W2t": w2t,
    }


def kernel(x, W0, W1, W2, _trace=False):
    from concourse.bass_utils import run_bass_kernel_spmd

    x = np.ascontiguousarray(x, dtype=np.float32)
    w0s, w1t, w2t = _pack_weights(W0, W1, W2)

    nc = _get_nc(B_CORE)
    in_maps = []
    for c in range(N_CORES):
        xc = x[c * B_CORE : (c + 1) * B_CORE]
        in_maps.append(_host_pack_core(xc, w0s, w1t, w2t))
    res = run_bass_kernel_spmd(
        nc, in_maps, core_ids=list(range(N_CORES)), trace=_trace
    )
    # per-core out: [3, H, B_CORE] -> [B_CORE, 3*H]
    outs = []
    for c in range(N_CORES):
        o = res.results[c]["out"]
        outs.append(o.reshape(3 * H, B_CORE).T.reshape(B_CORE, 3 * H))
    full = np.concatenate(outs, axis=0).astype(np.float32)
    if _trace:
        return full, res
    return full

